# revision 5
# baseline (speedup 1.0000x reference)
"""KimiMoE kernel for 8x TRN2 NeuronCores (Bass/Tile).

Strategy (expert-parallel, fp32):
  - Each core owns E/8 = 4 routed experts (weights sliced on host) and a
    1/8 slice of the shared-expert intermediate dim (256 of 2048).
  - Router is replicated: every core computes sigmoid(x@gate_w)+bias,
    top-8 via DVE max/max_index, and builds the dense combine matrix C.
    Expert columns are permuted per-core so each core's own 4 experts sit
    in columns 0..3 (keeps the SPMD program identical across cores);
    core 0's permutation is identity so its topk_ids are the real output.
  - Dense grouped-GEMM over token groups of TB=512 held in SBUF
    (xT transposed once per group via PE); 17 weight passes per group
    (1 shared + 4 experts x 4 I-quarters) stream weights through SBUF;
    output accumulates in SBUF, written once per group.
  - Partial outputs [T,H] are ReduceScattered across the 8 cores; each
    core emits its 1024-token slice; host concatenates.
"""

import sys

try:
    import concourse.bass as bass
except ImportError:  # fresh grading dir: repo not on sys.path
    sys.path.insert(0, "/opt/trn_rl_repo")
    import concourse.bass as bass

import numpy as np

from concourse import bacc
import concourse.mybir as mybir
from concourse.masks import make_identity
from concourse.tile import TileContext
from concourse.bass_utils import run_bass_kernel_spmd

F32 = mybir.dt.float32
U32 = mybir.dt.uint32
AF = mybir.ActivationFunctionType
ALU = mybir.AluOpType

T, H, E, K, I = 8192, 2048, 32, 8, 1024
NSH = 2
SCALE = 2.5
P = 128
NCORES = 8
EPC = E // NCORES            # experts per core
SH_I = NSH * I // NCORES     # shared intermediate slice per core
TB = 512                     # token group held in SBUF
NG = T // TB
IQ = 256                     # I-quarter streamed per pass (== SH_I)
NPASS = 1 + EPC * (I // IQ)  # shared + expert quarters


def build_nc():
    nc = bacc.Bacc("TRN2", num_devices=NCORES)

    x_d = nc.dram_tensor("x", [T, H], F32, kind="ExternalInput")
    gw_d = nc.dram_tensor("gate_w", [H, E], F32, kind="ExternalInput")
    gb_d = nc.dram_tensor("gate_bias_rep", [P, E], F32, kind="ExternalInput")
    wg_d = nc.dram_tensor("wg", [EPC, H, I], F32, kind="ExternalInput")
    wu_d = nc.dram_tensor("wu", [EPC, H, I], F32, kind="ExternalInput")
    wd_d = nc.dram_tensor("wd", [EPC, I, H], F32, kind="ExternalInput")
    sg_d = nc.dram_tensor("sg", [H, SH_I], F32, kind="ExternalInput")
    su_d = nc.dram_tensor("su", [H, SH_I], F32, kind="ExternalInput")
    sd_d = nc.dram_tensor("sd", [SH_I, H], F32, kind="ExternalInput")

    out_d = nc.dram_tensor("out_shard", [T // NCORES, H], F32, kind="ExternalOutput")
    ids_d = nc.dram_tensor("topk_ids", [T, K], U32, kind="ExternalOutput")

    part_d = nc.dram_tensor("part", [T, H], F32)
    rs_d = nc.dram_tensor("rs", [T // NCORES, H], F32)

    with TileContext(nc) as tc:
        with (
            tc.tile_pool(name="const", bufs=1) as constp,
            tc.tile_pool(name="xT", bufs=1) as xTp,
            tc.tile_pool(name="acc", bufs=1) as accp,
            tc.tile_pool(name="wgu", bufs=3) as wgup,
            tc.tile_pool(name="wdp", bufs=2) as wdp,
            tc.tile_pool(name="xin", bufs=2) as xinp,
            tc.tile_pool(name="sm", bufs=3) as smp,
            tc.tile_pool(name="rt", bufs=2) as rtp,
            tc.tile_pool(name="cg", bufs=1) as cgp,
            tc.tile_pool(name="ps_po", bufs=4, space="PSUM") as pop,
            tc.tile_pool(name="ps_h", bufs=2, space="PSUM") as hp,
            tc.tile_pool(name="ps_t", bufs=2, space="PSUM") as tp_p,
        ):
            ident = constp.tile([P, P], F32)
            make_identity(nc, ident)
            gwsb = constp.tile([P, H // P, E], F32)
            nc.sync.dma_start(gwsb[:], gw_d[:, :].rearrange("(o p) e -> p o e", p=P))
            biasb = constp.tile([P, E], F32)
            nc.sync.dma_start(biasb[:], gb_d[:, :])

            def group_body(g):
                t0g = g * TB
                xT = xTp.tile([P, H // P, TB], F32, tag="xT")
                Cg = cgp.tile([P, TB // P, E], F32, tag="Cg")
                acc = accp.tile([P, TB // P, H], F32, tag="acc")

                for j in range(TB // P):
                    t0 = t0g + j * P
                    xin = xinp.tile([P, H], F32, tag="xin")
                    nc.sync.dma_start(xin[:], x_d[bass.ds(t0, P), :])
                    for hk in range(H // P):
                        tp = tp_p.tile([P, P], F32, tag="tp")
                        nc.tensor.transpose(tp[:], xin[:, hk * P:(hk + 1) * P], ident[:])
                        nc.vector.tensor_copy(xT[:, hk, j * P:(j + 1) * P], tp[:])
                    # router for these 128 tokens
                    lg = tp_p.tile([P, E], F32, tag="tp")
                    for hk in range(H // P):
                        nc.tensor.matmul(
                            lg[:], xT[:, hk, j * P:(j + 1) * P], gwsb[:, hk, :],
                            start=(hk == 0), stop=(hk == H // P - 1),
                        )
                    scores = rtp.tile([P, E], F32, tag="sc")
                    nc.scalar.activation(scores[:], lg[:], AF.Sigmoid)
                    choice = rtp.tile([P, E], F32, tag="ch")
                    nc.vector.tensor_add(choice[:], scores[:], biasb[:])
                    mv = rtp.tile([P, K], F32, tag="mv")
                    nc.vector.max(mv[:], choice[:])
                    ids = rtp.tile([P, K], U32, tag="ids")
                    nc.vector.max_index(ids[:], mv[:], choice[:])
                    nc.sync.dma_start(ids_d[bass.ds(t0, P), :], ids[:])
                    mask = rtp.tile([P, E], F32, tag="mk")
                    nc.vector.tensor_scalar(mask[:], choice[:], mv[:, 7:8], None, op0=ALU.is_ge)
                    msc = rtp.tile([P, E], F32, tag="ms")
                    nc.vector.tensor_mul(msc[:], scores[:], mask[:])
                    den = rtp.tile([P, 1], F32, tag="dn")
                    nc.vector.reduce_sum(den[:], msc[:], axis=mybir.AxisListType.X)
                    nc.vector.tensor_scalar_add(den[:], den[:], 1e-20)
                    rden = rtp.tile([P, 1], F32, tag="rd")
                    nc.vector.reciprocal(rden[:], den[:])
                    nc.vector.tensor_scalar_mul(rden[:], rden[:], SCALE)
                    nc.vector.tensor_scalar_mul(Cg[:, j, :], msc[:], rden[:])

                for p_i in range(NPASS):
                    if p_i == 0:
                        wg_src, wu_src, wd_src, ce = sg_d[:, :], su_d[:, :], sd_d[:, :], None
                    else:
                        e, q = divmod(p_i - 1, I // IQ)
                        wg_src = wg_d[e, :, q * IQ:(q + 1) * IQ]
                        wu_src = wu_d[e, :, q * IQ:(q + 1) * IQ]
                        wd_src = wd_d[e, q * IQ:(q + 1) * IQ, :]
                        ce = e
                    wgt = wgup.tile([P, H // P, IQ], F32, tag="wgu")
                    nc.sync.dma_start(wgt[:], wg_src.rearrange("(o p) i -> p o i", p=P))
                    wut = wgup.tile([P, H // P, IQ], F32, tag="wgu")
                    nc.sync.dma_start(wut[:], wu_src.rearrange("(o p) i -> p o i", p=P))
                    wdt = wdp.tile([P, IQ // P, H], F32, tag="wd")
                    nc.sync.dma_start(wdt[:], wd_src.rearrange("(o p) h -> p o h", p=P))

                    for j in range(TB // P):
                        hg = hp.tile([P, IQ], F32, tag="h")
                        hu = hp.tile([P, IQ], F32, tag="h")
                        for hk in range(H // P):
                            nc.tensor.matmul(
                                hg[:], xT[:, hk, j * P:(j + 1) * P], wgt[:, hk, :],
                                start=(hk == 0), stop=(hk == H // P - 1),
                            )
                        for hk in range(H // P):
                            nc.tensor.matmul(
                                hu[:], xT[:, hk, j * P:(j + 1) * P], wut[:, hk, :],
                                start=(hk == 0), stop=(hk == H // P - 1),
                            )
                        hs = smp.tile([P, IQ], F32, tag="hs")
                        nc.scalar.activation(hs[:], hg[:], AF.Silu)
                        nc.vector.tensor_mul(hs[:], hs[:], hu[:])
                        if ce is not None:
                            nc.vector.tensor_scalar_mul(hs[:], hs[:], Cg[:, j, ce:ce + 1])
                        hsT = smp.tile([P, IQ], F32, tag="hsT")
                        for ik in range(IQ // P):
                            tp2 = tp_p.tile([P, P], F32, tag="tp")
                            nc.tensor.transpose(tp2[:], hs[:, ik * P:(ik + 1) * P], ident[:])
                            nc.vector.tensor_copy(hsT[:, ik * P:(ik + 1) * P], tp2[:])
                        for n in range(H // 512):
                            po = pop.tile([P, 512], F32, tag="po")
                            for ik in range(IQ // P):
                                nc.tensor.matmul(
                                    po[:], hsT[:, ik * P:(ik + 1) * P],
                                    wdt[:, ik, n * 512:(n + 1) * 512],
                                    start=(ik == 0), stop=(ik == IQ // P - 1),
                                )
                            dst = acc[:, j, n * 512:(n + 1) * 512]
                            if p_i == 0:
                                nc.scalar.copy(dst, po[:])
                            else:
                                nc.vector.tensor_add(dst, dst, po[:])

                for j in range(TB // P):
                    nc.sync.dma_start(
                        part_d[bass.ds(t0g + j * P, P), :], acc[:, j, :]
                    )

            with tc.For_i(0, NG, 1) as g:
                group_body(g)

    # Combine partial sums across cores; each core keeps its 1024-token slice.
    with (
        nc.semaphore("cc_sem") as cc_sem,
        nc.semaphore("fin_sem") as fin_sem,
        nc.Block() as blk,
    ):
        @blk.gpsimd
        def _(g):
            g.collective_compute(
                "ReduceScatter",
                ALU.add,
                replica_groups=[list(range(NCORES))],
                ins=[part_d.ap().opt()],
                outs=[rs_d.ap().opt()],
            ).then_inc(cc_sem)
            g.wait_ge(cc_sem, 1)
            g.dma_start(out=out_d[:, :], in_=rs_d[:, :]).then_inc(fin_sem, 16)
            g.wait_ge(fin_sem, 16)

    nc.finalize()
    return nc


_NC_CACHE = None


def _get_nc():
    global _NC_CACHE
    if _NC_CACHE is None:
        _NC_CACHE = build_nc()
    return _NC_CACHE


def kernel(hidden_states, gate_w, gate_bias, w_gate, w_up, w_down, sg_w, su_w, sd_w):
    x = np.ascontiguousarray(np.asarray(hidden_states, dtype=np.float32))
    gate_w = np.asarray(gate_w, dtype=np.float32)
    gate_bias = np.asarray(gate_bias, dtype=np.float32)
    w_gate = np.asarray(w_gate, dtype=np.float32)
    w_up = np.asarray(w_up, dtype=np.float32)
    w_down = np.asarray(w_down, dtype=np.float32)
    sg_w = np.asarray(sg_w, dtype=np.float32)
    su_w = np.asarray(su_w, dtype=np.float32)
    sd_w = np.asarray(sd_w, dtype=np.float32)

    in_maps = []
    for c in range(NCORES):
        own = list(range(c * EPC, (c + 1) * EPC))
        rest = [e for e in range(E) if e not in own]
        perm = own + rest  # core 0: identity
        in_maps.append({
            "x": x,
            "gate_w": np.ascontiguousarray(gate_w[:, perm]),
            "gate_bias_rep": np.ascontiguousarray(
                np.broadcast_to(gate_bias[perm], (P, E))
            ),
            "wg": np.ascontiguousarray(w_gate[own]),
            "wu": np.ascontiguousarray(w_up[own]),
            "wd": np.ascontiguousarray(w_down[own]),
            "sg": np.ascontiguousarray(sg_w[:, c * SH_I:(c + 1) * SH_I]),
            "su": np.ascontiguousarray(su_w[:, c * SH_I:(c + 1) * SH_I]),
            "sd": np.ascontiguousarray(sd_w[c * SH_I:(c + 1) * SH_I, :]),
        })

    global _last_in_maps
    _last_in_maps = in_maps
    res = run_bass_kernel_spmd(_get_nc(), in_maps, list(range(NCORES))).results
    out = np.concatenate([res[c]["out_shard"] for c in range(NCORES)], axis=0)
    ids = res[0]["topk_ids"].astype(np.int32)
    return out, ids


# revision 8
# speedup vs baseline: 1.2370x; 1.2370x over previous
"""KimiMoE kernel for 8x TRN2 NeuronCores (Bass/Tile).

Strategy (expert-parallel, fp32):
  - Each core owns E/8 = 4 routed experts (weights sliced on host) and a
    1/8 slice of the shared-expert intermediate dim (256 of 2048).
  - Router is replicated: every core computes sigmoid(x@gate_w)+bias,
    top-8 via DVE max/max_index, and builds the dense combine matrix C.
    Expert columns are permuted per-core so each core's own 4 experts sit
    in columns 0..3 (keeps the SPMD program identical across cores);
    core 0's permutation is identity so its topk_ids are the real output.
  - Dense grouped-GEMM over token groups of TB=512 held in SBUF
    (xT transposed once per group via PE); 17 weight passes per group
    (1 shared + 4 experts x 4 I-quarters) stream weights through SBUF;
    output accumulates in SBUF, written once per group.
  - Partial outputs [T,H] are ReduceScattered across the 8 cores; each
    core emits its 1024-token slice; host concatenates.
"""

import sys

try:
    import concourse.bass as bass
except ImportError:  # fresh grading dir: repo not on sys.path
    sys.path.insert(0, "/opt/trn_rl_repo")
    import concourse.bass as bass

import numpy as np

from concourse import bacc
import concourse.mybir as mybir
from concourse.masks import make_identity
from concourse.tile import TileContext
from concourse.bass_utils import run_bass_kernel_spmd

F32 = mybir.dt.float32
U32 = mybir.dt.uint32
AF = mybir.ActivationFunctionType
ALU = mybir.AluOpType

T, H, E, K, I = 8192, 2048, 32, 8, 1024
NSH = 2
SCALE = 2.5
P = 128
NCORES = 8
EPC = E // NCORES            # experts per core
SH_I = NSH * I // NCORES     # shared intermediate slice per core
TB = 512                     # token group held in SBUF
NG = T // TB
IQ = 256                     # I-quarter streamed per pass (== SH_I)
NPASS = 1 + EPC * (I // IQ)  # shared + expert quarters

import os
_ABLATE = os.environ.get("KMOE_ABLATE", "")


def build_nc():
    nc = bacc.Bacc("TRN2", num_devices=NCORES)

    x_d = nc.dram_tensor("x", [T, H], F32, kind="ExternalInput")
    gw_d = nc.dram_tensor("gate_w", [H, E], F32, kind="ExternalInput")
    gb_d = nc.dram_tensor("gate_bias_rep", [P, E], F32, kind="ExternalInput")
    wg_d = nc.dram_tensor("wg", [EPC, H, I], F32, kind="ExternalInput")
    wu_d = nc.dram_tensor("wu", [EPC, H, I], F32, kind="ExternalInput")
    wd_d = nc.dram_tensor("wd", [EPC, I, H], F32, kind="ExternalInput")
    sg_d = nc.dram_tensor("sg", [H, SH_I], F32, kind="ExternalInput")
    su_d = nc.dram_tensor("su", [H, SH_I], F32, kind="ExternalInput")
    sd_d = nc.dram_tensor("sd", [SH_I, H], F32, kind="ExternalInput")

    out_d = nc.dram_tensor("out_shard", [T // NCORES, H], F32, kind="ExternalOutput")
    ids_d = nc.dram_tensor("topk_ids", [T, K], U32, kind="ExternalOutput")

    part_d = nc.dram_tensor("part", [T, H], F32)
    rs_d = nc.dram_tensor("rs", [T // NCORES, H], F32)

    with TileContext(nc) as tc:
        with (
            tc.tile_pool(name="const", bufs=1) as constp,
            tc.tile_pool(name="xT", bufs=1) as xTp,
            tc.tile_pool(name="acc", bufs=1) as accp,
            tc.tile_pool(name="wgu", bufs=3) as wgup,
            tc.tile_pool(name="wdp", bufs=2) as wdp,
            tc.tile_pool(name="xin", bufs=2) as xinp,
            tc.tile_pool(name="sm", bufs=3) as smp,
            tc.tile_pool(name="rt", bufs=2) as rtp,
            tc.tile_pool(name="cg", bufs=1) as cgp,
            tc.tile_pool(name="ps_po", bufs=4, space="PSUM") as pop,
            tc.tile_pool(name="ps_h", bufs=2, space="PSUM") as hp,
            tc.tile_pool(name="ps_t", bufs=2, space="PSUM") as tp_p,
        ):
            ident = constp.tile([P, P], F32)
            make_identity(nc, ident)
            gwsb = constp.tile([P, H // P, E], F32)
            nc.sync.dma_start(gwsb[:], gw_d[:, :].rearrange("(o p) e -> p o e", p=P))
            biasb = constp.tile([P, E], F32)
            nc.sync.dma_start(biasb[:], gb_d[:, :])

            def group_body(g):
                t0g = g * TB
                xT = xTp.tile([P, H // P, TB], F32, tag="xT")
                Cg = cgp.tile([P, TB // P, E], F32, tag="Cg")
                acc = accp.tile([P, TB // P, H], F32, tag="acc")

                for j in range(TB // P):
                    t0 = t0g + j * P
                    xin = xinp.tile([P, H], F32, tag="xin")
                    nc.sync.dma_start(xin[:], x_d[bass.ds(t0, P), :])
                    for hk in range(H // P):
                        tp = tp_p.tile([P, P], F32, tag="tp")
                        nc.tensor.transpose(tp[:], xin[:, hk * P:(hk + 1) * P], ident[:])
                        nc.vector.tensor_copy(xT[:, hk, j * P:(j + 1) * P], tp[:])
                    # router for these 128 tokens
                    lg = tp_p.tile([P, E], F32, tag="tp")
                    for hk in range(H // P):
                        nc.tensor.matmul(
                            lg[:], xT[:, hk, j * P:(j + 1) * P], gwsb[:, hk, :],
                            start=(hk == 0), stop=(hk == H // P - 1),
                        )
                    scores = rtp.tile([P, E], F32, tag="sc")
                    nc.scalar.activation(scores[:], lg[:], AF.Sigmoid)
                    choice = rtp.tile([P, E], F32, tag="ch")
                    nc.vector.tensor_add(choice[:], scores[:], biasb[:])
                    mv = rtp.tile([P, K], F32, tag="mv")
                    nc.vector.max(mv[:], choice[:])
                    ids = rtp.tile([P, K], U32, tag="ids")
                    nc.vector.max_index(ids[:], mv[:], choice[:])
                    nc.sync.dma_start(ids_d[bass.ds(t0, P), :], ids[:])
                    mask = rtp.tile([P, E], F32, tag="mk")
                    nc.vector.tensor_scalar(mask[:], choice[:], mv[:, 7:8], None, op0=ALU.is_ge)
                    msc = rtp.tile([P, E], F32, tag="ms")
                    nc.vector.tensor_mul(msc[:], scores[:], mask[:])
                    den = rtp.tile([P, 1], F32, tag="dn")
                    nc.vector.reduce_sum(den[:], msc[:], axis=mybir.AxisListType.X)
                    nc.vector.tensor_scalar_add(den[:], den[:], 1e-20)
                    rden = rtp.tile([P, 1], F32, tag="rd")
                    nc.vector.reciprocal(rden[:], den[:])
                    nc.vector.tensor_scalar_mul(rden[:], rden[:], SCALE)
                    nc.vector.tensor_scalar_mul(Cg[:, j, :], msc[:], rden[:])

                npass = 1 if _ABLATE == "onepass" else NPASS
                for p_i in range(npass):
                    if p_i == 0:
                        wg_src, wu_src, wd_src, ce = sg_d[:, :], su_d[:, :], sd_d[:, :], None
                    else:
                        e, q = divmod(p_i - 1, I // IQ)
                        wg_src = wg_d[e, :, q * IQ:(q + 1) * IQ]
                        wu_src = wu_d[e, :, q * IQ:(q + 1) * IQ]
                        wd_src = wd_d[e, q * IQ:(q + 1) * IQ, :]
                        ce = e
                    wgt = wgup.tile([P, H // P, IQ], F32, tag="wgu")
                    nc.sync.dma_start(wgt[:], wg_src.rearrange("(o p) i -> p o i", p=P))
                    wut = wgup.tile([P, H // P, IQ], F32, tag="wgu")
                    nc.sync.dma_start(wut[:], wu_src.rearrange("(o p) i -> p o i", p=P))
                    wdt = wdp.tile([P, IQ // P, H], F32, tag="wd")
                    nc.sync.dma_start(wdt[:], wd_src.rearrange("(o p) h -> p o h", p=P))

                    for j in range(TB // P):
                        hg = hp.tile([P, IQ], F32, tag="h")
                        hu = hp.tile([P, IQ], F32, tag="h")
                        for hk in range(H // P):
                            nc.tensor.matmul(
                                hg[:], xT[:, hk, j * P:(j + 1) * P], wgt[:, hk, :],
                                start=(hk == 0), stop=(hk == H // P - 1),
                            )
                        for hk in range(H // P):
                            nc.tensor.matmul(
                                hu[:], xT[:, hk, j * P:(j + 1) * P], wut[:, hk, :],
                                start=(hk == 0), stop=(hk == H // P - 1),
                            )
                        hs = smp.tile([P, IQ], F32, tag="hs")
                        nc.scalar.activation(hs[:], hg[:], AF.Silu)
                        nc.vector.tensor_mul(hs[:], hs[:], hu[:])
                        if ce is not None:
                            nc.vector.tensor_scalar_mul(hs[:], hs[:], Cg[:, j, ce:ce + 1])
                        hsT = smp.tile([P, IQ], F32, tag="hsT")
                        for ik in range(IQ // P):
                            tp2 = tp_p.tile([P, P], F32, tag="tp")
                            nc.tensor.transpose(tp2[:], hs[:, ik * P:(ik + 1) * P], ident[:])
                            nc.vector.tensor_copy(hsT[:, ik * P:(ik + 1) * P], tp2[:])
                        for n in range(H // 512):
                            po = pop.tile([P, 512], F32, tag="po")
                            for ik in range(IQ // P):
                                nc.tensor.matmul(
                                    po[:], hsT[:, ik * P:(ik + 1) * P],
                                    wdt[:, ik, n * 512:(n + 1) * 512],
                                    start=(ik == 0), stop=(ik == IQ // P - 1),
                                )
                            dst = acc[:, j, n * 512:(n + 1) * 512]
                            if p_i == 0:
                                nc.scalar.copy(dst, po[:])
                            else:
                                nc.vector.tensor_add(dst, dst, po[:])

                for j in range(TB // P):
                    nc.sync.dma_start(
                        part_d[bass.ds(t0g + j * P, P), :], acc[:, j, :]
                    )

            with tc.For_i(0, NG, 1) as g:
                group_body(g)

    # Combine partial sums across cores; each core keeps its 1024-token slice.
    if _ABLATE == "nors":
        with (nc.semaphore("fin_sem") as fin_sem, nc.Block() as blk):
            @blk.gpsimd
            def _(g):
                g.dma_start(out=out_d[:, :], in_=part_d[: T // NCORES, :]).then_inc(fin_sem, 16)
                g.wait_ge(fin_sem, 16)
        nc.finalize()
        return nc
    with (
        nc.semaphore("cc_sem") as cc_sem,
        nc.semaphore("fin_sem") as fin_sem,
        nc.Block() as blk,
    ):
        @blk.gpsimd
        def _(g):
            g.collective_compute(
                "ReduceScatter",
                ALU.add,
                replica_groups=[list(range(NCORES))],
                ins=[part_d.ap().opt()],
                outs=[rs_d.ap().opt()],
            ).then_inc(cc_sem)
            g.wait_ge(cc_sem, 1)
            g.dma_start(out=out_d[:, :], in_=rs_d[:, :]).then_inc(fin_sem, 16)
            g.wait_ge(fin_sem, 16)

    nc.finalize()
    return nc


_NC_CACHE = None


def _get_nc():
    global _NC_CACHE
    if _NC_CACHE is None:
        _NC_CACHE = build_nc()
    return _NC_CACHE


def kernel(hidden_states, gate_w, gate_bias, w_gate, w_up, w_down, sg_w, su_w, sd_w):
    x = np.ascontiguousarray(np.asarray(hidden_states, dtype=np.float32))
    gate_w = np.asarray(gate_w, dtype=np.float32)
    gate_bias = np.asarray(gate_bias, dtype=np.float32)
    w_gate = np.asarray(w_gate, dtype=np.float32)
    w_up = np.asarray(w_up, dtype=np.float32)
    w_down = np.asarray(w_down, dtype=np.float32)
    sg_w = np.asarray(sg_w, dtype=np.float32)
    su_w = np.asarray(su_w, dtype=np.float32)
    sd_w = np.asarray(sd_w, dtype=np.float32)

    in_maps = []
    for c in range(NCORES):
        own = list(range(c * EPC, (c + 1) * EPC))
        rest = [e for e in range(E) if e not in own]
        perm = own + rest  # core 0: identity
        in_maps.append({
            "x": x,
            "gate_w": np.ascontiguousarray(gate_w[:, perm]),
            "gate_bias_rep": np.ascontiguousarray(
                np.broadcast_to(gate_bias[perm], (P, E))
            ),
            "wg": np.ascontiguousarray(w_gate[own]),
            "wu": np.ascontiguousarray(w_up[own]),
            "wd": np.ascontiguousarray(w_down[own]),
            "sg": np.ascontiguousarray(sg_w[:, c * SH_I:(c + 1) * SH_I]),
            "su": np.ascontiguousarray(su_w[:, c * SH_I:(c + 1) * SH_I]),
            "sd": np.ascontiguousarray(sd_w[c * SH_I:(c + 1) * SH_I, :]),
        })

    global _last_in_maps
    _last_in_maps = in_maps
    res = run_bass_kernel_spmd(_get_nc(), in_maps, list(range(NCORES))).results
    out = np.concatenate([res[c]["out_shard"] for c in range(NCORES)], axis=0)
    ids = res[0]["topk_ids"].astype(np.int32)
    return out, ids
